# revision 59
# baseline (speedup 1.0000x reference)
"""PointConv2 Trainium2 Bass kernel.

Data-parallel over B=8 across 8 NeuronCores (one batch element per core).

Per-core computation (feature F [64,16384] f32, idx [16384,8] i32 -> out [128,16384] f32):
  G  = relu(w2b @ relu(w2a @ F + b2a) + b2b)                 [64, N]
  gf[k*64+c, n]  = F[c, idx[n,k]]
  Gg[k*64+c', n] = G[c', idx[n,k]]
  L  = relu(w1b @ relu(w1a @ gf + b1a) + b1b)                [512, N]
  out = relu(w3b @ relu(w3a @ (Gg + L) + b3a) + b3b)         [128, N]

Strategy (v3 — ring-sized gathers, combined re-block tiles, linear tables):
  * Gathers are split into 1024-idx calls (= SWDGE descriptor-ring
    capacity) so the GpSimd engine never blocks mid-call and all 4 queue
    DSPs generate descriptors concurrently (~2ns/item aggregate).
    CRITICAL invariant: queue_num == 1-based issue ordinal mod 4.  The
    tile scheduler is a priority heap, so anything that makes one
    parity's idx list ready before the other reorders the gather stream
    and aliases a DMASW lane semaphore across two queues (both idx lists
    live in ONE I16 tile so their replication completes atomically; a
    post-compile guard asserts the scheduled queue rotation).
  * The post-gather PE transposes write into two combined channel-major
    tiles via partition-preserving half-copies: FCO = [F_odd; F_even]
    (single K=128 layer-1a contraction, no zero-padded weights) and
    GCO = [G_even; G_odd] (matches L's row order, full-partition S add).
  * Token tables are written LINEARLY (one contiguous 8KB descriptor per
    partition instead of 4096 interleaved 256B/128B row segments); the
    gather idx values are bit-swapped to the linear table address by a
    4-op i16 transform: addr(n) = ((n&127)<<5) | ((n>>7)&31) | (n&0x3000).
  * Phase 1 (G-mlp + tables) is software-pipelined with a 1-iteration
    skew: PE runs mm_a(nt), mm_b(nt-1) back to back; relu_a on scalar
    (activation), relu_b on vector (2-op tensor_scalar max(x+b, 0));
    f32->bf16 feature casts run as 1024-col halves on scalar between
    relus; idx re-pack copies fill vector slack; per 4096-token quarter
    an XBAR dma_start_transpose produces [F|G] rows, a vector swap-copy
    makes the [G|F] variant, and both stream out as contiguous writes
    split across the sync/scalar HWDGE queues.
  * Big weight preps run INSIDE the phase-2 pool scope: the phase-1
    pool-exit barrier would otherwise gate the first gather on them;
    issued after the warm-up gathers, they fill the PE's wait for the
    first gathered chunk.
  * Phase 2: per chunk of 1024 points, 4 even-k + 4 odd-k gather calls
    (tokd2 = [G|F] rows for even k, tokd = [F|G] for odd) with GBUF=4
    chunks in flight.  Gather idx lists are pre-arranged j-major: within
    a chunk, output column i = j*1024 + q (j = k-pair, q = ti*16 + p
    enumerates points n = c*1024 + p*64 + ti), so all layer-1a/1b/3a
    matmul rhs operands and the Gg+L adds are fully contiguous
    512-column slices.  The final 3b matmul un-permutes q -> n via a
    strided rhs access pattern so the output DMA is contiguous.
  * All matmuls bf16 (fp32 PSUM accumulate).
"""

import os

import numpy as np

QMULTI = int(os.environ.get("PC_QMULTI", "1"))

N = 16384
P = 128
CH = 1024          # points per gather chunk
NCH = N // CH      # 16
NIDX = CH * 4      # idxs per chunk per parity (4 k's x 1024 points)
GBUF = 4           # gather tiles in flight per parity
TBUF = 3           # re-blocked tiles in flight per parity

_cache = {}


def _build():
    if "nc" in _cache:
        return _cache["nc"]

    import concourse.bass as bass
    import concourse.mybir as mybir
    import concourse.tile as tile
    from concourse.bacc import Bacc
    from concourse.masks import make_identity

    f32 = mybir.dt.float32
    bf16 = mybir.dt.bfloat16
    i32 = mybir.dt.int32
    i16 = mybir.dt.int16
    RELU = mybir.ActivationFunctionType.Relu
    ADD = mybir.AluOpType.add
    MAX = mybir.AluOpType.max

    nc = Bacc("TRN2", target_bir_lowering=False, debug=False, num_devices=8,
              num_swdge_queues=4)

    feature = nc.dram_tensor("feature", [64, N], f32, kind="ExternalInput")
    idx = nc.dram_tensor("idx", [N, 8], i32, kind="ExternalInput")
    w1a = nc.dram_tensor("w1a", [512, 512], f32, kind="ExternalInput")
    b1a = nc.dram_tensor("b1a", [512], f32, kind="ExternalInput")
    w1b = nc.dram_tensor("w1b", [512, 512], f32, kind="ExternalInput")
    b1b = nc.dram_tensor("b1b", [512], f32, kind="ExternalInput")
    w2a = nc.dram_tensor("w2a", [128, 64], f32, kind="ExternalInput")
    b2a = nc.dram_tensor("b2a", [128], f32, kind="ExternalInput")
    w2b = nc.dram_tensor("w2b", [64, 128], f32, kind="ExternalInput")
    b2b = nc.dram_tensor("b2b", [64], f32, kind="ExternalInput")
    w3a = nc.dram_tensor("w3a", [256, 512], f32, kind="ExternalInput")
    b3a = nc.dram_tensor("b3a", [256], f32, kind="ExternalInput")
    w3b = nc.dram_tensor("w3b", [128, 256], f32, kind="ExternalInput")
    b3b = nc.dram_tensor("b3b", [128], f32, kind="ExternalInput")
    out = nc.dram_tensor("out", [128, N], f32, kind="ExternalOutput")
    tokd = nc.dram_tensor("tokd", [N, P], bf16, kind="Internal")
    tokd2 = nc.dram_tensor("tokd2", [N, P], bf16, kind="Internal")

    with tile.TileContext(nc) as tc:
        with (
            tc.tile_pool(name="const", bufs=1) as const,
            tc.tile_pool(name="idxp", bufs=1) as idxp,
            tc.tile_pool(name="psum", bufs=8, space="PSUM") as psum,
        ):
            idf = const.tile([P, P], f32)
            make_identity(nc, idf)
            idb = const.tile([P, P], bf16)
            make_identity(nc, idb)

            # ---- weights to bf16 lhsT layouts (via PE transpose) ----
            W1 = const.tile([P, 2048], bf16)    # [c][j*512+m]: p<64 -> w1a[m,(2j+1)*64+p], p>=64 -> w1a[m,(2j)*64+(p-64)]
            W1B = const.tile([P, 2048], bf16)   # [p][ci*512+m] = w1b[m, ci*128+p]
            W3A = const.tile([P, 1024], bf16)   # [p][ti*256+m] = w3a[m, ti*128+p]
            W3B = const.tile([P, 256], bf16)    # [p][ti*128+m] = w3b[m, ti*128+p]
            W2A = const.tile([P, 128], bf16)    # [c][m] = w2a[m, c] on partitions 0-63
            W2B = const.tile([P, 64], bf16)     # [p][m] = w2b[m, p]

            with tc.tile_pool(name="wtmp", bufs=2) as wtmp:
                # ---- small W2 + bias loads FIRST (ahead of the big feature
                # loads in the sync HWDGE queue): G-mlp start depends on them
                # ---- gather warm-up: 4 dummy 128-idx gathers (one per SWDGE
                # queue) pay the gather-ucode LIBRARY_RELOAD + first-call
                # overhead (~8us) here, while gpsimd is otherwise idle, instead
                # of on the critical path at the first real gather.  They are
                # pool-DMA ordinals 1-4, keeping the lane/queue rotation.
                DIDX = idxp.tile([P, 8], i16)
                nc.gpsimd.memset(DIDX, 0)
                DDST = idxp.tile([P, 1, 64], f32)
                dtab = feature.ap().rearrange("c (r e) -> (c r) e", e=64)
                for qd in range(4):
                    nc.gpsimd.dma_gather(
                        DDST, dtab, DIDX[:],
                        num_idxs=128, num_idxs_reg=128, elem_size=64,
                        transpose=False, single_packet=False,
                        queue_num=(qd + 1) % 4,
                    )

                nat5 = wtmp.tile([P, 64], f32, tag="wnat5", bufs=1)
                nc.sync.dma_start(nat5, w2a.ap())
                nat6 = wtmp.tile([64, 128], f32, tag="wnat6", bufs=1)
                nc.sync.dma_start(nat6, w2b.ap())
                B2A = const.tile([P, 1], f32)
                nc.sync.dma_start(B2A, b2a.ap()[:, None])
                B2B = const.tile([P, 1], f32)
                nc.sync.dma_start(B2B[64:128, :], b2b.ap()[:, None])

                # feature loads as f32 via HWDGE in rotating 2048-col slabs;
                # casts to bf16 are issued just-in-time inside the pipelined
                # G-loop (SWDGE cast-DMA would break the gather queue rotation)
                FG = wtmp.tile([P, N], bf16, tag="fg", bufs=1)
                ft_t = {}

                def load_slab(g):
                    ft_t[g] = wtmp.tile([64, 2048], f32, tag="ft32", bufs=4, name=f"ft{g}")
                    nc.sync.dma_start(ft_t[g], feature.ap()[:, g * 2048 : (g + 1) * 2048])

                for g in range(4):
                    load_slab(g)

                def cast_half(i):
                    # scalar, [64, 1024] halves: smooth 1us pieces interleave
                    # with relu_a instead of bursty 2us full-slab casts
                    cs = slice(i * 1024, i * 1024 + 1024)
                    nc.scalar.copy(FG[0:64, cs], ft_t[i // 2][:, (i % 2) * 1024 : (i % 2) * 1024 + 1024])

                pt = psum.tile([P, P], f32, tag="mm")
                nc.tensor.transpose(pt[0:64, :], nat5, idf)
                nc.vector.tensor_copy(W2A[0:64, :], pt[0:64, :])
                pt = psum.tile([P, P], f32, tag="mm")
                nc.tensor.transpose(pt[:, 0:64], nat6, idf[0:64, 0:64])
                nc.vector.tensor_copy(W2B, pt[:, 0:64])

                # ---- idx prep (j-major wrapped lists) ----
                # L32[p][c][(ti k)] = idx[c*1024 + p*64 + ti, k]; 2KB runs.
                # Within-chunk gather column becomes i = j*1024 + q with
                # q = ti*16 + p <-> point n = c*1024 + p*64 + ti.
                # The i16 re-pack runs as 32 single-c copies spread over the
                # G-loop (vector slack); replication to 128 partitions uses
                # log-doubling (3 DMAs per list instead of 7).
                L32 = wtmp.tile([16, 16, 512], i32, tag="i32", bufs=1)
                nc.sync.dma_start(
                    L32, idx.ap().rearrange("(c p ti) k -> p c (ti k)", c=16, p=16)
                )
                # E and O lists live in ONE tile so the doubling DMAs complete
                # atomically for both parities: the tile scheduler is a
                # priority heap, and if one parity's list became ready first,
                # it would hoist that parity's gathers, breaking the SWDGE
                # lane/queue rotation (queue must equal issue ordinal mod 4).
                I16 = idxp.tile([P, 2, 4096], i16)
                I16E = I16[:, 0, :]
                I16O = I16[:, 1, :]
                bit = L32[:].bitcast(i16)  # [16, 16, 1024 (ti k two)]
                # I16E[p][c*256 + j*64 + ti] = bit[p][c][ti*16 + 4j]
                bitv = bit.rearrange("p c (ti sixteen) -> p c sixteen ti", sixteen=16)
                I16Ev = I16[0:16, 0, :].rearrange("p (c j m) -> p c j m", c=16, j=4)
                I16Ov = I16[0:16, 1, :].rearrange("p (c j m) -> p c j m", c=16, j=4)

                def idx_subcopy(i):
                    # on vector: gpsimd is too slow here (~1.3us each) and its
                    # in-order queue would delay the first gather behind them.
                    # E/O alternate so both lists complete at the same time.
                    cs = slice(i // 2, i // 2 + 1)
                    if i % 2 == 0:
                        nc.vector.tensor_copy(I16Ev[:, cs], bitv[:, cs, 0:16:4, :])
                    else:
                        nc.vector.tensor_copy(I16Ov[:, cs], bitv[:, cs, 2:16:4, :])

                # token tables are written LINEARLY (write order == SBUF tile
                # order, one contiguous 8KB descriptor per partition), so the
                # idx VALUES are transformed to the bit-swapped table address:
                #   addr(n) = ((n&127)<<5) | ((n>>7)&31) | (n&0x3000)
                # (token n = q*4096 + r*128 + p lands at table row q*4096+p*32+r)
                SHL = mybir.AluOpType.logical_shift_left
                SHR = mybir.AluOpType.logical_shift_right
                AND = mybir.AluOpType.bitwise_and
                OR = mybir.AluOpType.bitwise_or
                TT1 = idxp.tile([16, 4096], i16)
                TT2 = idxp.tile([16, 4096], i16)

                def idx_transform(par, step):
                    v = I16[0:16, par, :]
                    if step == 0:
                        nc.vector.tensor_scalar(TT1, v, 5, 0x0FE0, SHL, AND)
                    elif step == 1:
                        nc.vector.tensor_scalar(TT2, v, 7, 0x001F, SHR, AND)
                    elif step == 2:
                        nc.vector.tensor_scalar(v, v, 0x3000, None, AND)
                        nc.vector.tensor_tensor(v, v, TT1, OR)
                    else:
                        nc.vector.tensor_tensor(v, v, TT2, OR)

                def idx_double(r):
                    # round r: partitions [16*2^r : 32*2^r) <- [0 : 16*2^r),
                    # both parities in one DMA (atomic readiness)
                    w = 16 << r
                    nc.scalar.dma_start(I16[w : 2 * w], I16[0:w])

                # ---- phase 1: G = mlp2(F) into FG[64:128], then per-quarter
                # XBAR transpose to token-major and DMA out to the DRAM tables.
                # Software-pipelined with a 1-iteration skew (mm_b trails mm_a)
                # so neither in-order engine queue blocks the other: PE runs
                # mm_a(nt), mm_b(nt-1) back to back; relu_a on scalar,
                # relu_b on vector (scalar_tensor_tensor max(x+b, 0)).
                g2a_t = {}

                def g_mm_a(nt):
                    cols = slice(nt * 512, nt * 512 + 512)
                    g2a_ps = psum.tile([P, 512], f32, tag="mm")
                    nc.tensor.matmul(g2a_ps, W2A[0:64, :], FG[0:64, cols], start=True, stop=True)
                    g2a_t[nt] = wtmp.tile([P, 512], bf16, tag="g2a", bufs=4, name=f"g2a{nt}")
                    nc.scalar.activation(g2a_t[nt], g2a_ps, RELU, bias=B2A)

                def g_mm_b(nt):
                    cols = slice(nt * 512, nt * 512 + 512)
                    ps = psum.tile([P, 512], f32, tag="mm")
                    nc.tensor.matmul(
                        ps[64:128, :], W2B, g2a_t[nt], start=True, stop=True,
                        tile_position=(0, 64),
                    )
                    nc.vector.tensor_scalar(
                        FG[64:128, cols], ps[64:128, :], B2B[64:128, :], 0.0,
                        ADD, MAX,
                    )

                def g_quarter_out(q):
                    # XBAR transpose to token-major STG ([F|G] rows), vector
                    # swap-copy to STG2 ([G|F]), then one fully-contiguous
                    # linear write per table (1 descriptor per partition).
                    # NOTE: splitting quarter 3 into two halves (to overlap
                    # write drain with the XBAR) measured ~+6us normalized —
                    # the halves queue behind q2's writes anyway and pay
                    # double issue overhead.  Keep whole-quarter granularity.
                    STG = wtmp.tile([P, 32, P], bf16, tag="stg", bufs=2, name=f"stg{q}")
                    nc.sync.dma_start_transpose(STG, FG[:, q * 4096 : (q + 1) * 4096])
                    STG2 = wtmp.tile([P, 32, P], bf16, tag="stg2", bufs=2, name=f"stg2{q}")
                    nc.vector.tensor_copy(STG2[:, :, 0:64], STG[:, :, 64:128])
                    nc.vector.tensor_copy(STG2[:, :, 64:128], STG[:, :, 0:64])
                    qrows = slice(q * 4096, (q + 1) * 4096)
                    dst1 = tokd.ap()[qrows, :].rearrange("(p r) e -> p (r e)", p=P)
                    dst2 = tokd2.ap()[qrows, :].rearrange("(p r) e -> p (r e)", p=P)
                    nc.sync.dma_start(dst1, STG.rearrange("p a b -> p (a b)"))
                    nc.scalar.dma_start(dst2, STG2.rearrange("p a b -> p (a b)"))

                for i in range(3):
                    cast_half(i)
                for nt in range(32):
                    g_mm_a(nt)
                    if nt % 4 == 2 and nt // 4 + 4 < 8:
                        load_slab(nt // 4 + 4)
                    if nt % 2 == 0 and nt // 2 + 3 < 16:
                        cast_half(nt // 2 + 3)
                    idx_subcopy(nt)
                    if nt > 0:
                        g_mm_b(nt - 1)
                        if nt % 8 == 0:
                            g_quarter_out(nt // 8 - 1)
                g_mm_b(31)
                g_quarter_out(3)
                for step in range(4):
                    idx_transform(0, step)
                for step in range(4):
                    idx_transform(1, step)
                for r in range(3):
                    idx_double(r)

                # ---- remaining biases (tiny; const pool outlives this scope)
                B1A = const.tile([P, 4], f32)
                nc.scalar.dma_start(B1A, b1a.ap().rearrange("(o p) -> p o", p=P))
                B1B = const.tile([P, 4], f32)
                nc.scalar.dma_start(B1B, b1b.ap().rearrange("(o p) -> p o", p=P))
                B3A = const.tile([P, 2], f32)
                nc.scalar.dma_start(B3A, b3a.ap().rearrange("(o p) -> p o", p=P))
                B3B = const.tile([P, 1], f32)
                nc.scalar.dma_start(B3B, b3b.ap()[:, None])

            # ---- phase 2 ----
            with (
                tc.tile_pool(name="gath", bufs=1) as gathp,
                tc.tile_pool(name="work", bufs=1) as workp,
                tc.tile_pool(name="wprep", bufs=2) as wtmp2,
            ):
              def weight_preps():
                  # big weight preps, issued inside the phase-2 scope so the
                  # phase-1 pool-exit barrier doesn't gate the first gather on
                  # them; the PE transposes fill the gather warm-up window.
                  # Loads go on scalar (its DMA queue is lighter than sync's).
                  nat1 = wtmp2.tile([P, 4, 512], f32, tag="wnat", bufs=1)
                  nc.scalar.dma_start(nat1, w1a.ap().rearrange("(ro p) c -> p ro c", p=P))
                  for co in range(4):
                      for ro in range(4):
                          pt = psum.tile([P, P], f32, tag="mm")
                          base = co * 128
                          nc.tensor.transpose(pt[0:64, :], nat1[:, ro, base + 64 : base + 128], idf)
                          # transpose-mode MMs must write PSUM partition 0; use a
                          # plain matmul against identity for the partition-64 half
                          nc.tensor.matmul(
                              pt[64:128, :], nat1[:, ro, base : base + 64], idf,
                              start=True, stop=True, tile_position=(0, 64),
                          )
                          nc.vector.tensor_copy(W1[:, co * 512 + ro * 128 : co * 512 + ro * 128 + 128], pt)

                  nat2 = wtmp2.tile([P, 4, 512], f32, tag="wnat", bufs=1)
                  nc.scalar.dma_start(nat2, w1b.ap().rearrange("(ro p) c -> p ro c", p=P))
                  for ci in range(4):
                      for mo in range(4):
                          pt = psum.tile([P, P], f32, tag="mm")
                          nc.tensor.transpose(pt, nat2[:, mo, ci * 128 : ci * 128 + 128], idf)
                          nc.vector.tensor_copy(W1B[:, ci * 512 + mo * 128 : ci * 512 + mo * 128 + 128], pt)

                  nat3 = wtmp2.tile([P, 2, 512], f32, tag="wnat3", bufs=1)
                  nc.scalar.dma_start(nat3, w3a.ap().rearrange("(ro p) c -> p ro c", p=P))
                  for ti in range(4):
                      for mo in range(2):
                          pt = psum.tile([P, P], f32, tag="mm")
                          nc.tensor.transpose(pt, nat3[:, mo, ti * 128 : ti * 128 + 128], idf)
                          nc.vector.tensor_copy(W3A[:, ti * 256 + mo * 128 : ti * 256 + mo * 128 + 128], pt)

                  nat4 = wtmp2.tile([P, 256], f32, tag="wnat4", bufs=1)
                  nc.scalar.dma_start(nat4, w3b.ap())
                  for ti in range(2):
                      pt = psum.tile([P, P], f32, tag="mm")
                      nc.tensor.transpose(pt, nat4[:, ti * 128 : ti * 128 + 128], idf)
                      nc.vector.tensor_copy(W3B[:, ti * 128 : ti * 128 + 128], pt)
              gte, gto, fco, gco = {}, {}, {}, {}
              _ordinal = [5]  # queue = ordinal mod 4; ordinals 1-4 are the
              # phase-1 dummy warm-up gathers

              def _gather(dst, tab, idx_ap, n):
                  q = _ordinal[0] % 4 if QMULTI else 0
                  _ordinal[0] += 1
                  nc.gpsimd.dma_gather(
                      dst, tab.ap(), idx_ap,
                      num_idxs=n, num_idxs_reg=n, elem_size=P,
                      transpose=False, single_packet=False, queue_num=q,
                  )

              def issue_gathers(c):
                  # ring-sized calls: the SWDGE ring holds 1024 descriptors, so
                  # a 1024-idx call never blocks the engine mid-generation and
                  # all 4 queues' desc-gen DSPs stay fed.  Chunk 0 uses finer
                  # 512-idx calls so the first re-block transposes (which only
                  # need the first block pair) start sooner.
                  gte[c] = gathp.tile([P, 32, P], bf16, tag="gte", bufs=GBUF, name=f"gte{c}")
                  gto[c] = gathp.tile([P, 32, P], bf16, tag="gto", bufs=GBUF, name=f"gto{c}")
                  nsub = 8 if c == 0 else 4
                  nb = 32 // nsub
                  ni = 256 // nsub
                  for i in range(nsub):
                      bs = slice(nb * i, nb * i + nb)
                      isl = slice(c * 256 + ni * i, c * 256 + ni * i + ni)
                      _gather(gte[c][:, bs, :], tokd2, I16E[:, isl], NIDX // nsub)
                      _gather(gto[c][:, bs, :], tokd, I16O[:, isl], NIDX // nsub)

              def issue_transposes(c):
                  # PE re-block to channel-major combined tiles.  Even token
                  # rows ([G|F] from tokd2) transpose to psum [G_e; F_e]; odd
                  # rows ([F|G]) to [F_o; G_o].  The four half-partition copies
                  # are all partition-preserving:
                  #   FCO = [F_o(0:64) ; F_e(64:128)]  (layer-1a rhs, matches W1)
                  #   GCO = [G_e(0:64) ; G_o(64:128)]  (Gg, matches L row order)
                  fco[c] = gathp.tile([P, 32, P], bf16, tag="fco", bufs=TBUF, name=f"fco{c}")
                  gco[c] = gathp.tile([P, 32, P], bf16, tag="gco", bufs=TBUF, name=f"gco{c}")
                  fcv = fco[c].rearrange("p a b -> p (a b)")
                  gcv = gco[c].rearrange("p a b -> p (a b)")
                  # NOTE: sb must stay in gather-arrival order (0..7) — an
                  # evens-first reorder (to land layer-1a h=0's copies sooner)
                  # measured ~10us SLOWER: transposes then stall on later
                  # gather calls while earlier blocks sit ready.
                  for sb in range(8):
                      pe_ = psum.tile([P, 512], bf16, tag="mm")
                      po_ = psum.tile([P, 512], bf16, tag="mm")
                      for u in range(4):
                          s_ = sb * 4 + u
                          us = slice(u * P, u * P + P)
                          nc.tensor.transpose(pe_[:, us], gte[c][:, s_, :], idb)
                          nc.tensor.transpose(po_[:, us], gto[c][:, s_, :], idb)
                      cs = slice(sb * 512, sb * 512 + 512)
                      # 2/2 scalar/vector split: with vector doing 3 of 4
                      # copies (1.2us/sb vs PE's 0.7us/sb transpose pace) the
                      # psum 8-buf rotation catches up and stalls PE ~930ns
                      # per chunk; scalar is idle in this window.
                      nc.vector.tensor_copy(fcv[0:64, cs], po_[0:64, :])
                      nc.scalar.copy(fcv[64:128, cs], pe_[64:128, :])
                      nc.scalar.copy(gcv[0:64, cs], pe_[0:64, :])
                      nc.vector.tensor_copy(gcv[64:128, cs], po_[64:128, :])

              def issue_compute(c):
                  # within-chunk column i = j*1024 + q, q = ti*16 + p16
                  #   <-> point n = c*1024 + p16*64 + ti
                  FCv = fco[c].rearrange("p a b -> p (a b)")
                  GCv = gco[c].rearrange("p a b -> p (a b)")

                  # layer 1a for both halves first, so the FCO tile is fully
                  # consumed early and the next chunk's transposes can overlap
                  # the 1b/3a/3b tail.  NOTE: keeping each psum's accumulation
                  # run contiguous (h OUTER) measures ~3us/chunk faster than
                  # interleaving two psum groups to share lhsT loads.
                  zr = {}
                  for h in range(2):
                      for o in range(4):
                          z1 = psum.tile([P, 512], f32, tag="mm")
                          for j in range(4):
                              cs = slice(j * 1024 + h * 512, j * 1024 + h * 512 + 512)
                              nc.tensor.matmul(
                                  z1, W1[:, j * 512 + o * 128 : j * 512 + o * 128 + 128],
                                  FCv[:, cs], start=(j == 0), stop=(j == 3),
                              )
                          t = workp.tile([P, 512], bf16, tag="zr", bufs=8, name=f"zr{h}{o}")
                          nc.scalar.activation(t, z1, RELU, bias=B1A[:, o : o + 1])
                          zr[h, o] = t
                  lr = {}
                  for h in range(2):
                      for o in range(4):
                          lps = psum.tile([P, 512], f32, tag="mm")
                          for ci in range(4):
                              nc.tensor.matmul(
                                  lps, W1B[:, ci * 512 + o * 128 : ci * 512 + o * 128 + 128],
                                  zr[h, ci], start=(ci == 0), stop=(ci == 3),
                              )
                          t = workp.tile([P, 512], bf16, tag="lr", bufs=8, name=f"lr{h}{o}")
                          nc.scalar.activation(t, lps, RELU, bias=B1B[:, o : o + 1])
                          lr[h, o] = t
                  # S = Gg + L: GCO partition order matches L's row order
                  S = {}
                  for h in range(2):
                      for t_ in range(4):
                          cs = slice(t_ * 1024 + h * 512, t_ * 1024 + h * 512 + 512)
                          st = workp.tile([P, 512], bf16, tag="s", bufs=8, name=f"s{h}{t_}")
                          nc.vector.tensor_tensor(st, GCv[:, cs], lr[h, t_], ADD)
                          S[h, t_] = st
                  zr3 = workp.tile([P, 2, CH], bf16, tag="zr3", bufs=2)
                  for h in range(2):
                      hs = slice(h * 512, h * 512 + 512)
                      for o3 in range(2):
                          z3 = psum.tile([P, 512], f32, tag="mm")
                          for t_ in range(4):
                              nc.tensor.matmul(
                                  z3, W3A[:, t_ * 256 + o3 * 128 : t_ * 256 + o3 * 128 + 128],
                                  S[h, t_], start=(t_ == 0), stop=(t_ == 3),
                              )
                          nc.scalar.activation(
                              zr3[:, o3, hs], z3, RELU, bias=B3A[:, o3 : o3 + 1],
                          )
                  # layer 3b: un-permute q = ti*16 + p16 -> n = p16*64 + ti via rhs AP
                  zr3v = zr3.rearrange("p t (ti sixteen) -> p t sixteen ti", sixteen=16)
                  for v in range(2):
                      ops = psum.tile([P, 512], f32, tag="mm")
                      for t_ in range(2):
                          rhs = zr3v[:, t_, 8 * v : 8 * v + 8, :]
                          nc.tensor.matmul(
                              ops, W3B[:, t_ * 128 : t_ * 128 + 128], rhs,
                              start=(t_ == 0), stop=(t_ == 1),
                          )
                      osb = workp.tile([P, 512], f32, tag="osb", bufs=2, name=f"osb{v}")
                      nc.scalar.activation(osb, ops, RELU, bias=B3B)
                      nc.scalar.dma_start(out.ap()[:, c * 1024 + v * 512 : c * 1024 + v * 512 + 512], osb)

              # startup: gathers first (gpsimd), then weight preps (PE fills
              # the gather warm-up window), then only transposes(0) before
              # compute(0) — more would make compute(0) wait (PE in-order) on
              # later gathers.  The ramp rebuilds the 2-chunk transpose lead.
              for c in range(min(GBUF, NCH)):
                  issue_gathers(c)
              weight_preps()
              issue_transposes(0)
              for c in range(NCH):
                  issue_compute(c)
                  if c == 0:
                      issue_transposes(1)
                  if c + 2 < NCH:
                      issue_transposes(c + 2)
                  if c + GBUF < NCH:
                      issue_gathers(c + GBUF)

    nc.compile()

    # Guard: the SWDGE lane<->queue discipline requires the gathers to stay in
    # emission order after tile scheduling (queue == 1-based ordinal mod 4,
    # DMASW lane == 0-based ordinal mod 8).  A reorder would alias one lane's
    # semaphore across two queues — a silent race on hardware.
    qs = []
    for fn in nc.m.functions:
        for bb in fn.blocks:
            for inst in bb.instructions:
                if type(inst).__name__ == "InstDMAGatherAnt":
                    qs.append(inst.queue_num)
    assert all(q == (i + 1) % 4 for i, q in enumerate(qs)), (
        "tile scheduler reordered the gather stream; SWDGE queue rotation "
        f"broken: {[(i, q) for i, q in enumerate(qs) if q != (i + 1) % 4][:8]}"
    )

    _cache["nc"] = nc
    return nc


def kernel(**inputs):
    from concourse import bass_utils

    nc = _build()
    feature = np.ascontiguousarray(inputs["feature"], dtype=np.float32)
    idx = np.ascontiguousarray(inputs["idx"], dtype=np.int32)
    weights = {
        k: np.ascontiguousarray(np.asarray(inputs[k]), dtype=np.float32)
        for k in ("w1a", "b1a", "w1b", "b1b", "w2a", "b2a", "w2b", "b2b",
                  "w3a", "b3a", "w3b", "b3b")
    }
    in_maps = []
    for b in range(8):
        m = {"feature": feature[b], "idx": idx[b]}
        m.update(weights)
        in_maps.append(m)
    res = bass_utils.run_bass_kernel_spmd(nc, in_maps, core_ids=list(range(8)))
    return np.stack([res.results[b]["out"] for b in range(8)]).astype(np.float32)



# revision 61
# speedup vs baseline: 1.0299x; 1.0299x over previous
"""PointConv2 Trainium2 Bass kernel.

Data-parallel over B=8 across 8 NeuronCores (one batch element per core).

Per-core computation (feature F [64,16384] f32, idx [16384,8] i32 -> out [128,16384] f32):
  G  = relu(w2b @ relu(w2a @ F + b2a) + b2b)                 [64, N]
  gf[k*64+c, n]  = F[c, idx[n,k]]
  Gg[k*64+c', n] = G[c', idx[n,k]]
  L  = relu(w1b @ relu(w1a @ gf + b1a) + b1b)                [512, N]
  out = relu(w3b @ relu(w3a @ (Gg + L) + b3a) + b3b)         [128, N]

Strategy (v3 — ring-sized gathers, combined re-block tiles, linear tables):
  * Gathers are split into 1024-idx calls (= SWDGE descriptor-ring
    capacity) so the GpSimd engine never blocks mid-call and all 4 queue
    DSPs generate descriptors concurrently (~2ns/item aggregate).
    CRITICAL invariant: queue_num == 1-based issue ordinal mod 4.  The
    tile scheduler is a priority heap, so anything that makes one
    parity's idx list ready before the other reorders the gather stream
    and aliases a DMASW lane semaphore across two queues (both idx lists
    live in ONE I16 tile so their replication completes atomically; a
    post-compile guard asserts the scheduled queue rotation).
  * The post-gather PE transposes write into two combined channel-major
    tiles via partition-preserving half-copies: FCO = [F_odd; F_even]
    (single K=128 layer-1a contraction, no zero-padded weights) and
    GCO = [G_even; G_odd] (matches L's row order, full-partition S add).
  * Token tables are written LINEARLY (one contiguous 8KB descriptor per
    partition instead of 4096 interleaved 256B/128B row segments); the
    gather idx values are bit-swapped to the linear table address by a
    4-op i16 transform: addr(n) = ((n&127)<<5) | ((n>>7)&31) | (n&0x3000).
  * Phase 1 (G-mlp + tables) is software-pipelined with a 1-iteration
    skew: PE runs mm_a(nt), mm_b(nt-1) back to back; relu_a on scalar
    (activation), relu_b on vector (2-op tensor_scalar max(x+b, 0));
    f32->bf16 feature casts run as 1024-col halves on scalar between
    relus; idx re-pack copies fill vector slack; per 4096-token quarter
    an XBAR dma_start_transpose produces [F|G] rows, a vector swap-copy
    makes the [G|F] variant, and both stream out as contiguous writes
    split across the sync/scalar HWDGE queues.
  * Big weight preps run INSIDE the phase-2 pool scope: the phase-1
    pool-exit barrier would otherwise gate the first gather on them;
    issued after the warm-up gathers, they fill the PE's wait for the
    first gathered chunk.
  * Phase 2: per chunk of 1024 points, 4 even-k + 4 odd-k gather calls
    (tokd2 = [G|F] rows for even k, tokd = [F|G] for odd) with GBUF=4
    chunks in flight.  Gather idx lists are pre-arranged j-major: within
    a chunk, output column i = j*1024 + q (j = k-pair, q = ti*16 + p
    enumerates points n = c*1024 + p*64 + ti), so all layer-1a/1b/3a
    matmul rhs operands and the Gg+L adds are fully contiguous
    512-column slices.  The final 3b matmul un-permutes q -> n via a
    strided rhs access pattern so the output DMA is contiguous.
  * All matmuls bf16 (fp32 PSUM accumulate).
"""

import os

import numpy as np

QMULTI = int(os.environ.get("PC_QMULTI", "1"))

N = 16384
P = 128
CH = 1024          # points per gather chunk
NCH = N // CH      # 16
NIDX = CH * 4      # idxs per chunk per parity (4 k's x 1024 points)
GBUF = 4           # gather tiles in flight per parity
TBUF = 3           # re-blocked tiles in flight per parity

_cache = {}


def _build():
    if "nc" in _cache:
        return _cache["nc"]

    import concourse.bass as bass
    import concourse.mybir as mybir
    import concourse.tile as tile
    from concourse.bacc import Bacc
    from concourse.masks import make_identity

    f32 = mybir.dt.float32
    bf16 = mybir.dt.bfloat16
    i32 = mybir.dt.int32
    i16 = mybir.dt.int16
    RELU = mybir.ActivationFunctionType.Relu
    ADD = mybir.AluOpType.add
    MAX = mybir.AluOpType.max

    nc = Bacc("TRN2", target_bir_lowering=False, debug=False, num_devices=8,
              num_swdge_queues=4)

    feature = nc.dram_tensor("feature", [64, N], f32, kind="ExternalInput")
    idx = nc.dram_tensor("idx", [N, 8], i32, kind="ExternalInput")
    w1a = nc.dram_tensor("w1a", [512, 512], f32, kind="ExternalInput")
    b1a = nc.dram_tensor("b1a", [512], f32, kind="ExternalInput")
    w1b = nc.dram_tensor("w1b", [512, 512], f32, kind="ExternalInput")
    b1b = nc.dram_tensor("b1b", [512], f32, kind="ExternalInput")
    w2a = nc.dram_tensor("w2a", [128, 64], f32, kind="ExternalInput")
    b2a = nc.dram_tensor("b2a", [128], f32, kind="ExternalInput")
    w2b = nc.dram_tensor("w2b", [64, 128], f32, kind="ExternalInput")
    b2b = nc.dram_tensor("b2b", [64], f32, kind="ExternalInput")
    w3a = nc.dram_tensor("w3a", [256, 512], f32, kind="ExternalInput")
    b3a = nc.dram_tensor("b3a", [256], f32, kind="ExternalInput")
    w3b = nc.dram_tensor("w3b", [128, 256], f32, kind="ExternalInput")
    b3b = nc.dram_tensor("b3b", [128], f32, kind="ExternalInput")
    out = nc.dram_tensor("out", [128, N], f32, kind="ExternalOutput")
    tokd = nc.dram_tensor("tokd", [N, P], bf16, kind="Internal")
    tokd2 = nc.dram_tensor("tokd2", [N, P], bf16, kind="Internal")

    with tile.TileContext(nc) as tc:
        with (
            tc.tile_pool(name="const", bufs=1) as const,
            tc.tile_pool(name="idxp", bufs=1) as idxp,
            tc.tile_pool(name="psum", bufs=8, space="PSUM") as psum,
        ):
            idf = const.tile([P, P], f32)
            make_identity(nc, idf)
            idb = const.tile([P, P], bf16)
            make_identity(nc, idb)

            # ---- weights to bf16 lhsT layouts (via PE transpose) ----
            W1 = const.tile([P, 2048], bf16)    # [c][j*512+m]: p<64 -> w1a[m,(2j+1)*64+p], p>=64 -> w1a[m,(2j)*64+(p-64)]
            W1B = const.tile([P, 2048], bf16)   # [p][ci*512+m] = w1b[m, ci*128+p]
            W3A = const.tile([P, 1024], bf16)   # [p][ti*256+m] = w3a[m, ti*128+p]
            W3B = const.tile([P, 256], bf16)    # [p][ti*128+m] = w3b[m, ti*128+p]
            W2A = const.tile([P, 128], bf16)    # [c][m] = w2a[m, c] on partitions 0-63
            W2B = const.tile([P, 64], bf16)     # [p][m] = w2b[m, p]

            with tc.tile_pool(name="wtmp", bufs=2) as wtmp:
                # ---- small W2 + bias loads FIRST (ahead of the big feature
                # loads in the sync HWDGE queue): G-mlp start depends on them
                # ---- gather warm-up: 4 dummy 128-idx gathers (one per SWDGE
                # queue) pay the gather-ucode LIBRARY_RELOAD + first-call
                # overhead (~8us) here, while gpsimd is otherwise idle, instead
                # of on the critical path at the first real gather.  They are
                # pool-DMA ordinals 1-4, keeping the lane/queue rotation.
                DIDX = idxp.tile([P, 8], i16)
                nc.gpsimd.memset(DIDX, 0)
                DDST = idxp.tile([P, 1, 64], f32)
                dtab = feature.ap().rearrange("c (r e) -> (c r) e", e=64)
                for qd in range(4):
                    nc.gpsimd.dma_gather(
                        DDST, dtab, DIDX[:],
                        num_idxs=128, num_idxs_reg=128, elem_size=64,
                        transpose=False, single_packet=False,
                        queue_num=(qd + 1) % 4,
                    )

                nat5 = wtmp.tile([P, 64], f32, tag="wnat5", bufs=1)
                nc.sync.dma_start(nat5, w2a.ap())
                nat6 = wtmp.tile([64, 128], f32, tag="wnat6", bufs=1)
                nc.sync.dma_start(nat6, w2b.ap())
                B2A = const.tile([P, 1], f32)
                nc.sync.dma_start(B2A, b2a.ap()[:, None])
                B2B = const.tile([P, 1], f32)
                nc.sync.dma_start(B2B[64:128, :], b2b.ap()[:, None])

                # feature loads as f32 via HWDGE in rotating 2048-col slabs;
                # casts to bf16 are issued just-in-time inside the pipelined
                # G-loop (SWDGE cast-DMA would break the gather queue rotation)
                FG = wtmp.tile([P, N], bf16, tag="fg", bufs=1)
                ft_t = {}

                def load_slab(g):
                    ft_t[g] = wtmp.tile([64, 2048], f32, tag="ft32", bufs=4, name=f"ft{g}")
                    nc.sync.dma_start(ft_t[g], feature.ap()[:, g * 2048 : (g + 1) * 2048])

                for g in range(4):
                    load_slab(g)

                def cast_half(i):
                    # scalar, [64, 1024] halves: smooth 1us pieces interleave
                    # with relu_a instead of bursty 2us full-slab casts
                    cs = slice(i * 1024, i * 1024 + 1024)
                    nc.scalar.copy(FG[0:64, cs], ft_t[i // 2][:, (i % 2) * 1024 : (i % 2) * 1024 + 1024])

                pt = psum.tile([P, P], f32, tag="mm")
                nc.tensor.transpose(pt[0:64, :], nat5, idf)
                nc.vector.tensor_copy(W2A[0:64, :], pt[0:64, :])
                pt = psum.tile([P, P], f32, tag="mm")
                nc.tensor.transpose(pt[:, 0:64], nat6, idf[0:64, 0:64])
                nc.vector.tensor_copy(W2B, pt[:, 0:64])

                # ---- idx prep (j-major wrapped lists) ----
                # L32[p][c][(ti k)] = idx[c*1024 + p*64 + ti, k]; 2KB runs.
                # Within-chunk gather column becomes i = j*1024 + q with
                # q = ti*16 + p <-> point n = c*1024 + p*64 + ti.
                # The i16 re-pack runs as 32 single-c copies spread over the
                # G-loop (vector slack); replication to 128 partitions uses
                # log-doubling (3 DMAs per list instead of 7).
                L32 = wtmp.tile([16, 16, 512], i32, tag="i32", bufs=1)
                nc.sync.dma_start(
                    L32, idx.ap().rearrange("(c p ti) k -> p c (ti k)", c=16, p=16)
                )
                # E and O lists live in ONE tile so the doubling DMAs complete
                # atomically for both parities: the tile scheduler is a
                # priority heap, and if one parity's list became ready first,
                # it would hoist that parity's gathers, breaking the SWDGE
                # lane/queue rotation (queue must equal issue ordinal mod 4).
                I16 = idxp.tile([P, 2, 4096], i16)
                I16E = I16[:, 0, :]
                I16O = I16[:, 1, :]
                bit = L32[:].bitcast(i16)  # [16, 16, 1024 (ti k two)]
                # I16E[p][c*256 + j*64 + ti] = bit[p][c][ti*16 + 4j]
                bitv = bit.rearrange("p c (ti sixteen) -> p c sixteen ti", sixteen=16)
                I16Ev = I16[0:16, 0, :].rearrange("p (c j m) -> p c j m", c=16, j=4)
                I16Ov = I16[0:16, 1, :].rearrange("p (c j m) -> p c j m", c=16, j=4)

                def idx_subcopy(i):
                    # on vector: gpsimd is too slow here (~1.3us each) and its
                    # in-order queue would delay the first gather behind them.
                    # E/O alternate so both lists complete at the same time.
                    cs = slice(i // 2, i // 2 + 1)
                    if i % 2 == 0:
                        nc.vector.tensor_copy(I16Ev[:, cs], bitv[:, cs, 0:16:4, :])
                    else:
                        nc.vector.tensor_copy(I16Ov[:, cs], bitv[:, cs, 2:16:4, :])

                # token tables are written LINEARLY (write order == SBUF tile
                # order, one contiguous 8KB descriptor per partition), so the
                # idx VALUES are transformed to the bit-swapped table address:
                #   addr(n) = ((n&127)<<5) | ((n>>7)&31) | (n&0x3000)
                # (token n = q*4096 + r*128 + p lands at table row q*4096+p*32+r)
                SHL = mybir.AluOpType.logical_shift_left
                SHR = mybir.AluOpType.logical_shift_right
                AND = mybir.AluOpType.bitwise_and
                OR = mybir.AluOpType.bitwise_or
                TT1 = idxp.tile([16, 4096], i16)
                TT2 = idxp.tile([16, 4096], i16)

                def idx_transform(par, step):
                    v = I16[0:16, par, :]
                    if step == 0:
                        nc.vector.tensor_scalar(TT1, v, 5, 0x0FE0, SHL, AND)
                    elif step == 1:
                        nc.vector.tensor_scalar(TT2, v, 7, 0x001F, SHR, AND)
                    elif step == 2:
                        nc.vector.tensor_scalar(v, v, 0x3000, None, AND)
                        nc.vector.tensor_tensor(v, v, TT1, OR)
                    else:
                        nc.vector.tensor_tensor(v, v, TT2, OR)

                def idx_double(r):
                    # round r: partitions [16*2^r : 32*2^r) <- [0 : 16*2^r),
                    # both parities in one DMA (atomic readiness)
                    w = 16 << r
                    nc.scalar.dma_start(I16[w : 2 * w], I16[0:w])

                # ---- phase 1: G = mlp2(F) into FG[64:128], then per-quarter
                # XBAR transpose to token-major and DMA out to the DRAM tables.
                # Software-pipelined with a 1-iteration skew (mm_b trails mm_a)
                # so neither in-order engine queue blocks the other: PE runs
                # mm_a(nt), mm_b(nt-1) back to back; relu_a on scalar,
                # relu_b on vector (scalar_tensor_tensor max(x+b, 0)).
                g2a_t = {}

                def g_mm_a(nt):
                    cols = slice(nt * 512, nt * 512 + 512)
                    g2a_ps = psum.tile([P, 512], f32, tag="mm")
                    nc.tensor.matmul(g2a_ps, W2A[0:64, :], FG[0:64, cols], start=True, stop=True)
                    g2a_t[nt] = wtmp.tile([P, 512], bf16, tag="g2a", bufs=4, name=f"g2a{nt}")
                    nc.scalar.activation(g2a_t[nt], g2a_ps, RELU, bias=B2A)

                def g_mm_b(nt):
                    cols = slice(nt * 512, nt * 512 + 512)
                    ps = psum.tile([P, 512], f32, tag="mm")
                    nc.tensor.matmul(
                        ps[64:128, :], W2B, g2a_t[nt], start=True, stop=True,
                        tile_position=(0, 64),
                    )
                    nc.vector.tensor_scalar(
                        FG[64:128, cols], ps[64:128, :], B2B[64:128, :], 0.0,
                        ADD, MAX,
                    )

                def g_quarter_out(q):
                    # XBAR transpose to token-major STG ([F|G] rows), vector
                    # swap-copy to STG2 ([G|F]), then one fully-contiguous
                    # linear write per table (1 descriptor per partition).
                    # NOTE: splitting quarter 3 into two halves (to overlap
                    # write drain with the XBAR) measured ~+6us normalized —
                    # the halves queue behind q2's writes anyway and pay
                    # double issue overhead.  Keep whole-quarter granularity.
                    STG = wtmp.tile([P, 32, P], bf16, tag="stg", bufs=2, name=f"stg{q}")
                    nc.sync.dma_start_transpose(STG, FG[:, q * 4096 : (q + 1) * 4096])
                    STG2 = wtmp.tile([P, 32, P], bf16, tag="stg2", bufs=2, name=f"stg2{q}")
                    nc.vector.tensor_copy(STG2[:, :, 0:64], STG[:, :, 64:128])
                    nc.vector.tensor_copy(STG2[:, :, 64:128], STG[:, :, 0:64])
                    qrows = slice(q * 4096, (q + 1) * 4096)
                    dst1 = tokd.ap()[qrows, :].rearrange("(p r) e -> p (r e)", p=P)
                    dst2 = tokd2.ap()[qrows, :].rearrange("(p r) e -> p (r e)", p=P)
                    nc.sync.dma_start(dst1, STG.rearrange("p a b -> p (a b)"))
                    nc.scalar.dma_start(dst2, STG2.rearrange("p a b -> p (a b)"))

                for i in range(3):
                    cast_half(i)
                for nt in range(32):
                    g_mm_a(nt)
                    if nt % 4 == 2 and nt // 4 + 4 < 8:
                        load_slab(nt // 4 + 4)
                    if nt % 2 == 0 and nt // 2 + 3 < 16:
                        cast_half(nt // 2 + 3)
                    idx_subcopy(nt)
                    if nt > 0:
                        g_mm_b(nt - 1)
                        if nt % 8 == 0:
                            g_quarter_out(nt // 8 - 1)
                g_mm_b(31)
                g_quarter_out(3)
                for step in range(4):
                    idx_transform(0, step)
                for step in range(4):
                    idx_transform(1, step)
                for r in range(3):
                    idx_double(r)

                # ---- remaining biases (tiny; const pool outlives this scope)
                B1A = const.tile([P, 4], f32)
                nc.scalar.dma_start(B1A, b1a.ap().rearrange("(o p) -> p o", p=P))
                B1B = const.tile([P, 4], f32)
                nc.scalar.dma_start(B1B, b1b.ap().rearrange("(o p) -> p o", p=P))
                B3A = const.tile([P, 2], f32)
                nc.scalar.dma_start(B3A, b3a.ap().rearrange("(o p) -> p o", p=P))
                B3B = const.tile([P, 1], f32)
                nc.scalar.dma_start(B3B, b3b.ap()[:, None])

            # ---- phase 2 ----
            with (
                tc.tile_pool(name="gath", bufs=1) as gathp,
                tc.tile_pool(name="work", bufs=1) as workp,
                tc.tile_pool(name="wprep", bufs=2) as wtmp2,
            ):
              def weight_preps():
                  # big weight preps, issued inside the phase-2 scope so the
                  # phase-1 pool-exit barrier doesn't gate the first gather on
                  # them; the PE transposes fill the gather warm-up window.
                  # Loads go on scalar (its DMA queue is lighter than sync's).
                  nat1 = wtmp2.tile([P, 4, 512], f32, tag="wnat", bufs=1)
                  nc.scalar.dma_start(nat1, w1a.ap().rearrange("(ro p) c -> p ro c", p=P))
                  for co in range(4):
                      for ro in range(4):
                          pt = psum.tile([P, P], f32, tag="mm")
                          base = co * 128
                          nc.tensor.transpose(pt[0:64, :], nat1[:, ro, base + 64 : base + 128], idf)
                          # transpose-mode MMs must write PSUM partition 0; use a
                          # plain matmul against identity for the partition-64 half
                          nc.tensor.matmul(
                              pt[64:128, :], nat1[:, ro, base : base + 64], idf,
                              start=True, stop=True, tile_position=(0, 64),
                          )
                          nc.vector.tensor_copy(W1[:, co * 512 + ro * 128 : co * 512 + ro * 128 + 128], pt)

                  nat2 = wtmp2.tile([P, 4, 512], f32, tag="wnat", bufs=1)
                  nc.scalar.dma_start(nat2, w1b.ap().rearrange("(ro p) c -> p ro c", p=P))
                  for ci in range(4):
                      for mo in range(4):
                          pt = psum.tile([P, P], f32, tag="mm")
                          nc.tensor.transpose(pt, nat2[:, mo, ci * 128 : ci * 128 + 128], idf)
                          nc.vector.tensor_copy(W1B[:, ci * 512 + mo * 128 : ci * 512 + mo * 128 + 128], pt)

                  nat3 = wtmp2.tile([P, 2, 512], f32, tag="wnat3", bufs=1)
                  nc.scalar.dma_start(nat3, w3a.ap().rearrange("(ro p) c -> p ro c", p=P))
                  for ti in range(4):
                      for mo in range(2):
                          pt = psum.tile([P, P], f32, tag="mm")
                          nc.tensor.transpose(pt, nat3[:, mo, ti * 128 : ti * 128 + 128], idf)
                          nc.vector.tensor_copy(W3A[:, ti * 256 + mo * 128 : ti * 256 + mo * 128 + 128], pt)

                  nat4 = wtmp2.tile([P, 256], f32, tag="wnat4", bufs=1)
                  nc.scalar.dma_start(nat4, w3b.ap())
                  for ti in range(2):
                      pt = psum.tile([P, P], f32, tag="mm")
                      nc.tensor.transpose(pt, nat4[:, ti * 128 : ti * 128 + 128], idf)
                      nc.vector.tensor_copy(W3B[:, ti * 128 : ti * 128 + 128], pt)
              gte, gto, fco, gco = {}, {}, {}, {}
              _ordinal = [5]  # queue = ordinal mod 4; ordinals 1-4 are the
              # phase-1 dummy warm-up gathers

              def _gather(dst, tab, idx_ap, n):
                  q = _ordinal[0] % 4 if QMULTI else 0
                  _ordinal[0] += 1
                  nc.gpsimd.dma_gather(
                      dst, tab.ap(), idx_ap,
                      num_idxs=n, num_idxs_reg=n, elem_size=P,
                      transpose=False, single_packet=False, queue_num=q,
                  )

              def issue_gathers(c):
                  # ring-sized calls: the SWDGE ring holds 1024 descriptors, so
                  # a 1024-idx call never blocks the engine mid-generation and
                  # all 4 queues' desc-gen DSPs stay fed.  Chunk 0 uses finer
                  # 512-idx calls so the first re-block transposes (which only
                  # need the first block pair) start sooner.
                  gte[c] = gathp.tile([P, 32, P], bf16, tag="gte", bufs=GBUF, name=f"gte{c}")
                  gto[c] = gathp.tile([P, 32, P], bf16, tag="gto", bufs=GBUF, name=f"gto{c}")
                  nsub = 8 if c == 0 else 4
                  nb = 32 // nsub
                  ni = 256 // nsub
                  for i in range(nsub):
                      bs = slice(nb * i, nb * i + nb)
                      isl = slice(c * 256 + ni * i, c * 256 + ni * i + ni)
                      _gather(gte[c][:, bs, :], tokd2, I16E[:, isl], NIDX // nsub)
                      _gather(gto[c][:, bs, :], tokd, I16O[:, isl], NIDX // nsub)

              def issue_transposes(c):
                  # PE re-block to channel-major combined tiles.  Even token
                  # rows ([G|F] from tokd2) transpose to psum [G_e; F_e]; odd
                  # rows ([F|G]) to [F_o; G_o].  The four half-partition copies
                  # are all partition-preserving:
                  #   FCO = [F_o(0:64) ; F_e(64:128)]  (layer-1a rhs, matches W1)
                  #   GCO = [G_e(0:64) ; G_o(64:128)]  (Gg, matches L row order)
                  fco[c] = gathp.tile([P, 32, P], bf16, tag="fco", bufs=TBUF, name=f"fco{c}")
                  gco[c] = gathp.tile([P, 32, P], bf16, tag="gco", bufs=TBUF, name=f"gco{c}")
                  fcv = fco[c].rearrange("p a b -> p (a b)")
                  gcv = gco[c].rearrange("p a b -> p (a b)")
                  # NOTE: sb must stay in gather-arrival order (0..7) — an
                  # evens-first reorder (to land layer-1a h=0's copies sooner)
                  # measured ~10us SLOWER: transposes then stall on later
                  # gather calls while earlier blocks sit ready.
                  for sb in range(8):
                      # both parities in ONE full-bank psum tile (even [G|F]
                      # at cols 0:512, odd [F|G] at 512:1024): halves the
                      # allocation count in the psum rotation that stalls PE
                      # ~930ns/chunk when the trailing copies back it up
                      pp = psum.tile([P, 1024], bf16, tag="mm")
                      for u in range(4):
                          s_ = sb * 4 + u
                          us = slice(u * P, u * P + P)
                          us2 = slice(512 + u * P, 512 + u * P + P)
                          nc.tensor.transpose(pp[:, us], gte[c][:, s_, :], idb)
                          nc.tensor.transpose(pp[:, us2], gto[c][:, s_, :], idb)
                      cs = slice(sb * 512, sb * 512 + 512)
                      # 1/3 scalar/vector split is measured-optimal: a 2/2
                      # split cost ~12us normalized — the extra scalar copies
                      # delay its relu chains more than the stall saves.
                      nc.vector.tensor_copy(fcv[0:64, cs], pp[0:64, 512:1024])
                      nc.scalar.copy(fcv[64:128, cs], pp[64:128, 0:512])
                      nc.vector.tensor_copy(gcv[0:64, cs], pp[0:64, 0:512])
                      nc.vector.tensor_copy(gcv[64:128, cs], pp[64:128, 512:1024])

              def issue_compute(c):
                  # within-chunk column i = j*1024 + q, q = ti*16 + p16
                  #   <-> point n = c*1024 + p16*64 + ti
                  FCv = fco[c].rearrange("p a b -> p (a b)")
                  GCv = gco[c].rearrange("p a b -> p (a b)")

                  # layer 1a for both halves first, so the FCO tile is fully
                  # consumed early and the next chunk's transposes can overlap
                  # the 1b/3a/3b tail.  NOTE: keeping each psum's accumulation
                  # run contiguous (h OUTER) measures ~3us/chunk faster than
                  # interleaving two psum groups to share lhsT loads.
                  zr = {}
                  for h in range(2):
                      for o in range(4):
                          z1 = psum.tile([P, 512], f32, tag="mm")
                          for j in range(4):
                              cs = slice(j * 1024 + h * 512, j * 1024 + h * 512 + 512)
                              nc.tensor.matmul(
                                  z1, W1[:, j * 512 + o * 128 : j * 512 + o * 128 + 128],
                                  FCv[:, cs], start=(j == 0), stop=(j == 3),
                              )
                          t = workp.tile([P, 512], bf16, tag="zr", bufs=8, name=f"zr{h}{o}")
                          nc.scalar.activation(t, z1, RELU, bias=B1A[:, o : o + 1])
                          zr[h, o] = t
                  lr = {}
                  for h in range(2):
                      for o in range(4):
                          lps = psum.tile([P, 512], f32, tag="mm")
                          for ci in range(4):
                              nc.tensor.matmul(
                                  lps, W1B[:, ci * 512 + o * 128 : ci * 512 + o * 128 + 128],
                                  zr[h, ci], start=(ci == 0), stop=(ci == 3),
                              )
                          t = workp.tile([P, 512], bf16, tag="lr", bufs=8, name=f"lr{h}{o}")
                          nc.scalar.activation(t, lps, RELU, bias=B1B[:, o : o + 1])
                          lr[h, o] = t
                  # S = Gg + L: GCO partition order matches L's row order
                  S = {}
                  for h in range(2):
                      for t_ in range(4):
                          cs = slice(t_ * 1024 + h * 512, t_ * 1024 + h * 512 + 512)
                          st = workp.tile([P, 512], bf16, tag="s", bufs=8, name=f"s{h}{t_}")
                          nc.vector.tensor_tensor(st, GCv[:, cs], lr[h, t_], ADD)
                          S[h, t_] = st
                  zr3 = workp.tile([P, 2, CH], bf16, tag="zr3", bufs=2)
                  for h in range(2):
                      hs = slice(h * 512, h * 512 + 512)
                      for o3 in range(2):
                          z3 = psum.tile([P, 512], f32, tag="mm")
                          for t_ in range(4):
                              nc.tensor.matmul(
                                  z3, W3A[:, t_ * 256 + o3 * 128 : t_ * 256 + o3 * 128 + 128],
                                  S[h, t_], start=(t_ == 0), stop=(t_ == 3),
                              )
                          nc.scalar.activation(
                              zr3[:, o3, hs], z3, RELU, bias=B3A[:, o3 : o3 + 1],
                          )
                  # layer 3b: un-permute q = ti*16 + p16 -> n = p16*64 + ti via rhs AP
                  zr3v = zr3.rearrange("p t (ti sixteen) -> p t sixteen ti", sixteen=16)
                  for v in range(2):
                      ops = psum.tile([P, 512], f32, tag="mm")
                      for t_ in range(2):
                          rhs = zr3v[:, t_, 8 * v : 8 * v + 8, :]
                          nc.tensor.matmul(
                              ops, W3B[:, t_ * 128 : t_ * 128 + 128], rhs,
                              start=(t_ == 0), stop=(t_ == 1),
                          )
                      osb = workp.tile([P, 512], f32, tag="osb", bufs=2, name=f"osb{v}")
                      nc.scalar.activation(osb, ops, RELU, bias=B3B)
                      nc.scalar.dma_start(out.ap()[:, c * 1024 + v * 512 : c * 1024 + v * 512 + 512], osb)

              # startup: gathers first (gpsimd), then weight preps (PE fills
              # the gather warm-up window), then only transposes(0) before
              # compute(0) — more would make compute(0) wait (PE in-order) on
              # later gathers.  The ramp rebuilds the 2-chunk transpose lead.
              for c in range(min(GBUF, NCH)):
                  issue_gathers(c)
              weight_preps()
              issue_transposes(0)
              for c in range(NCH):
                  issue_compute(c)
                  if c == 0:
                      issue_transposes(1)
                  if c + 2 < NCH:
                      issue_transposes(c + 2)
                  if c + GBUF < NCH:
                      issue_gathers(c + GBUF)

    nc.compile()

    # Guard: the SWDGE lane<->queue discipline requires the gathers to stay in
    # emission order after tile scheduling (queue == 1-based ordinal mod 4,
    # DMASW lane == 0-based ordinal mod 8).  A reorder would alias one lane's
    # semaphore across two queues — a silent race on hardware.
    qs = []
    for fn in nc.m.functions:
        for bb in fn.blocks:
            for inst in bb.instructions:
                if type(inst).__name__ == "InstDMAGatherAnt":
                    qs.append(inst.queue_num)
    assert all(q == (i + 1) % 4 for i, q in enumerate(qs)), (
        "tile scheduler reordered the gather stream; SWDGE queue rotation "
        f"broken: {[(i, q) for i, q in enumerate(qs) if q != (i + 1) % 4][:8]}"
    )

    _cache["nc"] = nc
    return nc


def kernel(**inputs):
    from concourse import bass_utils

    nc = _build()
    feature = np.ascontiguousarray(inputs["feature"], dtype=np.float32)
    idx = np.ascontiguousarray(inputs["idx"], dtype=np.int32)
    weights = {
        k: np.ascontiguousarray(np.asarray(inputs[k]), dtype=np.float32)
        for k in ("w1a", "b1a", "w1b", "b1b", "w2a", "b2a", "w2b", "b2b",
                  "w3a", "b3a", "w3b", "b3b")
    }
    in_maps = []
    for b in range(8):
        m = {"feature": feature[b], "idx": idx[b]}
        m.update(weights)
        in_maps.append(m)
    res = bass_utils.run_bass_kernel_spmd(nc, in_maps, core_ids=list(range(8)))
    return np.stack([res.results[b]["out"] for b in range(8)]).astype(np.float32)

